# revision 13
# baseline (speedup 1.0000x reference)
"""CourierEncoder fused kernel for 8 Trainium2 NeuronCores.

Data-parallel over the batch: each core processes B/8 = 32768 rows.
Per 512-row tile:
  embeds:  3 concurrent K=1 f32r matmuls (w (x) coord) on PE quadrants
           q0/q32/q64, cos folded as Sin(z+pi/2); ACT applies
           Sin/Sin/Prelu with exact per-partition f32 biases
  layer 1: feature-major, 6 bf16 matmuls [128,128]@[128,512];
           chunk0 act on ACT (fused bias Prelu), chunk1 on DVE
           (tensor_scalar add + scalar_tensor_tensor lrelu)
  layer 2: batch-major, b2 seeded by 2 K=1 ones-matmuls at q96, then
           8 bf16 matmuls accumulate; LeakyReLU via one DVE
           scalar_tensor_tensor (max(0.01*z, z)) straight from PSUM
  output stored as bf16 (upcast to f32 on host) -> halves HBM writes
"""

import math

import numpy as np
import ml_dtypes

import concourse.bass as bass
import concourse.tile as tile
import concourse.mybir as mybir
from concourse import bacc
from concourse.bass_utils import run_bass_kernel_spmd

B = 262144
NCORES = 8
R = B // NCORES          # rows per core
TILE = 512               # rows per tile
NT = R // TILE           # tiles per core
PED = 256
NED = 128
CED = 256
Q = PED // 4             # 64
ALPHA = 0.01

F32 = mybir.dt.float32
F32R = mybir.dt.float32r
BF16 = mybir.dt.bfloat16
AF = mybir.ActivationFunctionType
ALU = mybir.AluOpType

_CACHE = {}


def _build():
    nc = bacc.Bacc()
    xy = nc.dram_tensor("xyb", [R, 2], BF16, kind="ExternalInput")
    t = nc.dram_tensor("tb", [R, 1], BF16, kind="ExternalInput")
    emb_w = nc.dram_tensor("emb_w", [3, 128], BF16, kind="ExternalInput")
    biases = nc.dram_tensor("biases", [128, 5], F32, kind="ExternalInput")
    w1p = nc.dram_tensor("w1p", [128, 3, 2, 128], BF16, kind="ExternalInput")
    w2p = nc.dram_tensor("w2p", [128, 2, 256], BF16, kind="ExternalInput")
    b2row = nc.dram_tensor("b2row", [1, 512], BF16, kind="ExternalInput")
    out = nc.dram_tensor("out", [R, 256], BF16, kind="ExternalOutput")

    with tile.TileContext(nc) as tc:
        with (
            tc.tile_pool(name="const", bufs=1) as const,
            tc.tile_pool(name="io", bufs=4) as io,
            tc.tile_pool(name="emb", bufs=2) as emb,
            tc.tile_pool(name="h1", bufs=2) as h1,
            tc.tile_pool(name="outp", bufs=4) as outp,
            tc.tile_pool(name="ps_emb", bufs=1, space="PSUM") as ps_emb,
            tc.tile_pool(name="ps_l1", bufs=1, space="PSUM") as ps_l1,
            tc.tile_pool(name="ps_l2", bufs=1, space="PSUM") as ps_l2,
        ):
            emb_w_sb = const.tile([65, 128], BF16)
            bias_sb = const.tile([128, 5], F32)
            w1_sb = const.tile([128, 3, 2, 128], BF16)
            w2_sb = const.tile([128, 2, 256], BF16)
            # ones | b2row rows for the two L2 bias matmuls (q0 and q32)
            ob_q0 = const.tile([1, 640], BF16)
            ob_q32 = const.tile([33, 640], BF16)
            for c in range(3):
                nc.sync.dma_start(out=emb_w_sb[32 * c:32 * c + 1, :],
                                  in_=emb_w[c:c + 1, :])
            nc.sync.dma_start(out=bias_sb, in_=biases[:, :])
            nc.sync.dma_start(out=w1_sb, in_=w1p[:, :, :, :])
            nc.sync.dma_start(out=w2_sb, in_=w2p[:, :, :])
            nc.sync.dma_start(out=ob_q0[0:1, 128:640], in_=b2row[:, :])
            nc.sync.dma_start(out=ob_q32[32:33, 128:640], in_=b2row[:, :])
            nc.vector.memset(ob_q0[0:1, 0:128], 1.0)
            nc.vector.memset(ob_q32[32:33, 0:128], 1.0)

            for it in range(NT):
                base = it * TILE
                # -- load coords transposed, bf16 (gpsimd queue, no cast) ---
                xyt_b = io.tile([65, TILE], BF16)
                nc.gpsimd.dma_start(
                    out=xyt_b[0:1, :],
                    in_=xy[base:base + TILE, 0:1].rearrange("n c -> c n"),
                )
                nc.gpsimd.dma_start(
                    out=xyt_b[32:33, :],
                    in_=xy[base:base + TILE, 1:2].rearrange("n c -> c n"),
                )
                nc.gpsimd.dma_start(
                    out=xyt_b[64:65, :],
                    in_=t[base:base + TILE, :].rearrange("n c -> c n"),
                )

                # -- embeddings: 3 concurrent K=1 outer products ------------
                emb_ps = ps_emb.tile([128, 3, TILE], F32)
                for c in range(3):
                    nc.tensor.matmul(
                        emb_ps[:, c, :],
                        emb_w_sb[32 * c:32 * c + 1, :],
                        xyt_b[32 * c:32 * c + 1, :],
                        start=True, stop=True,
                    )
                hT = emb.tile([128, 3, TILE], BF16)
                nc.scalar.activation(out=hT[:, 0, :], in_=emb_ps[:, 0, :],
                                     func=AF.Sin, bias=bias_sb[:, 0:1])
                nc.scalar.activation(out=hT[:, 1, :], in_=emb_ps[:, 1, :],
                                     func=AF.Sin, bias=bias_sb[:, 1:2])
                nc.scalar.activation(out=hT[:, 2, :], in_=emb_ps[:, 2, :],
                                     func=AF.Prelu, bias=bias_sb[:, 2:3],
                                     alpha=ALPHA)

                # -- layer 1 (feature-major) --------------------------------
                l1_ps = ps_l1.tile([128, 2, TILE], F32)
                for mc in range(2):
                    for kc in range(3):
                        nc.tensor.matmul(
                            l1_ps[:, mc, :],
                            w1_sb[:, kc, mc, :],
                            hT[:, kc, :],
                            start=(kc == 0), stop=(kc == 2),
                        )
                h1T = h1.tile([128, 2, TILE], BF16)
                s0 = h1.tile([128, TILE], BF16, tag="s0")
                s1 = h1.tile([128, TILE], BF16, tag="s1")
                # both chunks: bias-add + lrelu on DVE (ACT is busier)
                nc.vector.tensor_scalar_add(out=s0, in0=l1_ps[:, 0, :],
                                            scalar1=bias_sb[:, 3:4])
                nc.vector.scalar_tensor_tensor(
                    out=h1T[:, 0, :], in0=s0, scalar=ALPHA, in1=s0,
                    op0=ALU.mult, op1=ALU.max,
                )
                nc.vector.tensor_scalar_add(out=s1, in0=l1_ps[:, 1, :],
                                            scalar1=bias_sb[:, 4:5])
                nc.vector.scalar_tensor_tensor(
                    out=h1T[:, 1, :], in0=s1, scalar=ALPHA, in1=s1,
                    op0=ALU.mult, op1=ALU.max,
                )

                # -- layer 2 (batch-major), emitted bank-by-bank ------------
                # Each [128,2,256] PSUM bank: bias ones-matmul seeds b2,
                # 4 matmuls accumulate, then one DVE stt drains it. The
                # drain of bank 0 overlaps the matmuls of bank 1, so
                # bufs=1 does not serialize consecutive tiles.
                l2_ps = ps_l2.tile([128, 4, 256], F32, tag="l2")
                o_sb = outp.tile([128, 4, 256], BF16)
                for half in range(2):
                    ob = ob_q0 if half == 0 else ob_q32
                    prow = 0 if half == 0 else 32
                    nc.tensor.matmul(
                        l2_ps[:, 2 * half:2 * half + 2, :],
                        ob[prow:prow + 1, 0:128],
                        ob[prow:prow + 1, 128:640],
                        start=True, stop=False,
                        skip_group_check=True,
                    )
                    for r in range(2 * half, 2 * half + 2):
                        for kc in range(2):
                            nc.tensor.matmul(
                                l2_ps[:, r, :],
                                h1T[:, kc, r * 128:(r + 1) * 128],
                                w2_sb[:, kc, :],
                                start=False, stop=(kc == 1),
                                skip_group_check=True,
                            )
                    # b2 is already in PSUM via the ones-matmul, so the
                    # activation is a bare Prelu -> one fused ACT op per bank
                    nc.scalar.activation(
                        out=o_sb[:, 2 * half:2 * half + 2, :],
                        in_=l2_ps[:, 2 * half:2 * half + 2, :],
                        func=AF.Prelu, bias=0.0, alpha=ALPHA,
                    )
                nc.sync.dma_start(
                    out=out[base:base + TILE, :].rearrange("(r p) m -> p r m", p=128),
                    in_=o_sb,
                )
    nc.finalize()
    return nc


def _prep_weights(inputs):
    f = {k: np.asarray(v, dtype=np.float32) for k, v in inputs.items()}
    bf = ml_dtypes.bfloat16

    emb_w = np.stack([
        np.concatenate([f["w_sx"].ravel(), f["w_cx"].ravel()]),
        np.concatenate([f["w_sy"].ravel(), f["w_cy"].ravel()]),
        f["w_t"].ravel(),
    ]).astype(bf)

    biases = np.zeros((128, 5), np.float32)
    biases[:, 0] = np.concatenate([f["b_sx"], f["b_cx"] + math.pi / 2])
    biases[:, 1] = np.concatenate([f["b_sy"], f["b_cy"] + math.pi / 2])
    biases[:, 2] = f["b_t"]
    biases[:, 3] = f["b1"][0:128]
    biases[:, 4] = f["b1"][128:256]

    w1p = f["w1"].reshape(3, 128, 2, 128).transpose(1, 0, 2, 3).astype(bf)
    w2p = f["w2"].reshape(2, 128, 256).transpose(1, 0, 2).astype(bf)

    b2row = np.concatenate([f["b2"], f["b2"]]).reshape(1, 512).astype(bf)

    return {
        "emb_w": emb_w,
        "biases": biases,
        "w1p": np.ascontiguousarray(w1p),
        "w2p": np.ascontiguousarray(w2p),
        "b2row": b2row,
    }


def kernel(**inputs):
    if "nc" not in _CACHE:
        _CACHE["nc"] = _build()
    nc = _CACHE["nc"]

    w = _prep_weights(inputs)
    bf = ml_dtypes.bfloat16
    xyb = np.ascontiguousarray(np.asarray(inputs["xy"]).astype(bf))
    tb = np.ascontiguousarray(np.asarray(inputs["t"]).astype(bf))

    in_maps = []
    for c in range(NCORES):
        lo, hi = c * R, (c + 1) * R
        in_maps.append({
            "xyb": xyb[lo:hi], "tb": tb[lo:hi], **w,
        })

    res = run_bass_kernel_spmd(nc, in_maps, core_ids=list(range(NCORES)))
    _CACHE["last_res"] = res
    return np.concatenate(
        [np.asarray(res.results[c]["out"]).astype(np.float32)
         for c in range(NCORES)], axis=0)
